# revision 16
# baseline (speedup 1.0000x reference)
"""ContrastiveDist kernel for TRN2 (8 NeuronCores, SPMD) -- v4.

out[n] = sum_e -(t_e . v_n) / (||t_e|| * ||v_n|| + eps)
       = (s . v_n) / ||v_n||      with s = -sum_e t_e / ||t_e||
(eps shifts the result by ~4e-11 relative -- dropped.)

v4 schedule fixes (from the v3 trace: DMA idle until 8.6us, phase-A
serial chain not done until 30us, PE cold-clocked, output DMA at 42us):
 * ALL input DMA issues first, split across BOTH HWDGE rings (SP via
   nc.sync + ACT via nc.scalar), ~0.65us per DMA_DIRECT2D overlapped
   with transfers.  eye rides inside the vt tensor (cols 0-48), no
   separate tensor.
 * target ships in natural entity-major layout [128e, 16, 256d]; per
   quarter (4 tiles): DVE TTR square+accum -> ACT Rsqrt -> DVE
   negate+bf16 -> PE matmuls with 1-column weights accumulate the s ROW
   [1, 256] in psum.  s is ready ~8us (vs 30us in v3).
 * s row -> per-half columns via two K=1 matmuls against a ones[1,1]
   rhs; dotw = eye * s_col broadcast (block-diag lhsT routes group g's
   [1,448] reduction to psum row g as in v3).
 * ONE ACT table set (sqrt_and_others: sqrt + square) covers phase-A
   sqrt, node-square offload, and both pair tails -> a single
   ACT_TABLE_LOAD at t~2us.  (ACT Rsqrt/Reciprocal are blocked by bass
   for accuracy; tails use ACT Sqrt + DVE reciprocal as in v3.)
 * node squares split DVE/ACT per chunk to keep both ahead of the PE
   ssq matmuls; final chunks are 1 group per ring so the post-last-byte
   chain (square -> 2 ssq mm -> rsqrt -> mul -> out DMA) is minimal.
 * pair tails: ACT Rsqrt [7,448] then one DVE mul, out DMAs on the SP
   ring.
"""

import os
import numpy as np
import ml_dtypes
from contextlib import ExitStack

# debug bisect toggles (default: full v4)
SP_DMA = bool(int(os.environ.get("V4_SP_DMA", "0")))      # all DMA on SP ring
BATCH_SSQ = bool(int(os.environ.get("V4_BATCH_SSQ", "0")))  # no fused TTR
NO_COLMM = bool(int(os.environ.get("V4_NO_COLMM", "0")))  # skip K=1 matmuls
SQ_DVE = bool(int(os.environ.get("V4_SQ_DVE", "0")))      # all squares on DVE

import concourse.bacc as bacc
import concourse.bass as bass
import concourse.mybir as mybir
import concourse.tile as tile
from concourse import bass_utils

E, D = 2048, 256          # entities, embed dim
N_FULL = 50000            # total nodes
N_CORES = 8
NPC = N_FULL // N_CORES   # 6250 true nodes per core
G = 448                   # node columns per psum group (fp32 bank width)
NG = 14                   # groups per core -> NPAD = 6272
NPAD = G * NG
NP = 7                    # groups per psum pair
A = 2                     # d-halves (256 = 2*128 partitions)
ET = E // 128             # 16 entity tiles [128, 256]
EYC = NP * NP             # eye columns folded into vt (49)
NC2 = EYC + NPAD          # vt total columns
TQ = 4                    # tgt DMA chunks (quarters of 4 tiles)
WARM_MM = 6               # PE prewarm dummy matmuls

# node chunks: (name, ring, [groups], square engine)
# ring S=SP(sync), A=ACT(scalar); square engine V=DVE, S=ACT
CHUNKS = [
    ("SA", "S", [0, 1, 2, 3], "V"),
    ("XA", "A", [4, 5, 6], "S"),
    ("SB", "S", [7, 8], "S"),
    ("XB", "A", [9, 10, 11], "V"),
    ("XC", "A", [12], "S"),
    ("SC", "S", [13], "V"),
]
# PE emission order for dots / ssq (by expected data-ready time)
DOT_ORDER = [4, 5, 6, 0, 1, 2, 3, 7, 8, 9, 10, 11, 12, 13]
SSQ_ORDER = [4, 5, 6, 0, 1, 2, 3, 9, 10, 11, 7, 8, 12, 13]

F32 = mybir.dt.float32
BF16 = mybir.dt.bfloat16
BF = ml_dtypes.bfloat16
SQRT = mybir.ActivationFunctionType.Sqrt
SQUARE = mybir.ActivationFunctionType.Square

_cache = {}


def _build():
    nc = bacc.Bacc(
        "TRN2",
        target_bir_lowering=False,
        debug=False,
        enable_asserts=True,
        num_devices=N_CORES,
    )
    tgt = nc.dram_tensor("target", [E, D], BF16, kind="ExternalInput").ap()
    vt = nc.dram_tensor("vt", [D, NC2], BF16, kind="ExternalInput").ap()
    out = nc.dram_tensor("out", [NG * G], F32, kind="ExternalOutput").ap()

    with tile.TileContext(nc) as tc, ExitStack() as ctx:
        tpool = ctx.enter_context(tc.tile_pool(name="tgt", bufs=1))
        vpool = ctx.enter_context(tc.tile_pool(name="v", bufs=1))
        spool = ctx.enter_context(tc.tile_pool(name="small", bufs=1))
        scr = ctx.enter_context(tc.tile_pool(name="scr", bufs=1))
        ps_w = ctx.enter_context(tc.tile_pool(name="psw", bufs=1, space="PSUM"))
        ps_sr = ctx.enter_context(tc.tile_pool(name="pssr", bufs=1, space="PSUM"))
        ps_c0 = ctx.enter_context(tc.tile_pool(name="psc0", bufs=1, space="PSUM"))
        ps_c1 = ctx.enter_context(tc.tile_pool(name="psc1", bufs=1, space="PSUM"))
        ps_da = ctx.enter_context(tc.tile_pool(name="psda", bufs=1, space="PSUM"))
        ps_db = ctx.enter_context(tc.tile_pool(name="psdb", bufs=1, space="PSUM"))
        ps_qa = ctx.enter_context(tc.tile_pool(name="psqa", bufs=1, space="PSUM"))
        ps_qb = ctx.enter_context(tc.tile_pool(name="psqb", bufs=1, space="PSUM"))

        tgt_sb = tpool.tile([128, ET, D], BF16, name="tgt_sb")
        tsq = scr.tile([128, ET, D], BF16, name="tsq")
        vt_sb = vpool.tile([128, A, NC2], BF16, name="vt_sb")
        vsq = vpool.tile([128, A, NPAD], BF16, name="vsq")

        ssq_t = spool.tile([128, ET], F32, name="ssq_t")
        tn = spool.tile([128, ET], F32, name="tn")
        inv_t = spool.tile([128, ET], F32, name="inv_t")
        winv = spool.tile([128, ET], BF16, name="winv")
        s_bf = spool.tile([1, D], BF16, name="s_bf")
        one_bf = spool.tile([1, 1], BF16, name="one_bf")
        s_colbf = spool.tile([128, A], BF16, name="s_colbf")
        dotw = spool.tile([128, A, EYC], BF16, name="dotw")
        warm_w = spool.tile([128, 1], BF16, name="warm_w")
        warm_x = spool.tile([128, G], BF16, name="warm_x")
        act_d = spool.tile([1, 1], F32, name="act_d")
        act_s = spool.tile([1, 1], F32, name="act_s")
        vn = [
            spool.tile([NP, G], F32, name="vna"),
            spool.tile([NP, G], F32, name="vnb"),
        ]
        isv = [
            spool.tile([NP, G], F32, name="isva"),
            spool.tile([NP, G], F32, name="isvb"),
        ]
        res = [
            spool.tile([NP, G], F32, name="resa"),
            spool.tile([NP, G], F32, name="resb"),
        ]

        warm_ps = ps_w.tile([1, G], F32, name="warm_ps")
        srow_ps = ps_sr.tile([1, D], F32, name="srow_ps")
        scol_ps = [
            ps_c0.tile([128, 1], F32, name="scol0"),
            ps_c1.tile([128, 1], F32, name="scol1"),
        ]
        dot_ps = [
            ps_da.tile([NP, G], F32, name="dot_psa"),
            ps_db.tile([NP, G], F32, name="dot_psb"),
        ]
        sq_ps = [
            ps_qa.tile([NP, G], F32, name="sq_psa"),
            ps_qb.tile([NP, G], F32, name="sq_psb"),
        ]

        tgt_v = tgt.rearrange("(p j) d -> p j d", j=ET)
        vt_v = vt.rearrange("(a p) n -> p a n", p=128)
        out_v = out.rearrange("(g f) -> g f", f=G)
        eye2d = vt_sb[:, 0, 0:EYC]           # [128, 49] block-diag bf16

        def vcols(g0, g1):
            return slice(EYC + g0 * G, EYC + g1 * G)

        # ---- memsets (gpsimd/vector, free) before anything
        nc.vector.memset(warm_w[:], 1.0)
        nc.vector.memset(warm_x[:], 0.0)
        nc.vector.memset(one_bf[:], 1.0)
        nc.vector.memset(act_d[:], 1.0)

        # ---- ALL input DMA issues first
        # SP ring: tgt quarters 0-1, eye, then SP node chunks
        H = ET // TQ
        for q in range(2):
            nc.sync.dma_start(
                tgt_sb[:, q * H : (q + 1) * H, :], tgt_v[:, q * H : (q + 1) * H, :]
            )
        nc.sync.dma_start(vt_sb[:, :, 0:EYC], vt_v[:, :, 0:EYC])
        # ACT ring: tgt quarters 2-3
        act_dma = nc.sync if SP_DMA else nc.scalar
        for q in range(2, 4):
            act_dma.dma_start(
                tgt_sb[:, q * H : (q + 1) * H, :], tgt_v[:, q * H : (q + 1) * H, :]
            )
        # ACT table preload (sqrt_and_others: sqrt + square)
        nc.scalar.activation(act_s[:], act_d[:], SQRT)
        # node chunks on their rings
        for name, ring, gs, _sq in CHUNKS:
            sl = vcols(gs[0], gs[-1] + 1)
            eng = nc.sync if (ring == "S" or SP_DMA) else nc.scalar
            eng.dma_start(vt_sb[:, :, sl], vt_v[:, :, sl])

        # ---- PE prewarm (HAM clock gate wants ~3.4us of activity)
        for _ in range(WARM_MM):
            nc.tensor.matmul(warm_ps[:], warm_w[:], warm_x[:], start=True, stop=True)

        # ---- phase A: s row = -sum_e t_e/||t_e||  as [1, 256] psum
        for q in range(TQ):
            j0 = q * H
            sl = slice(j0, j0 + H)
            if BATCH_SSQ:
                nc.vector.tensor_mul(
                    tsq[:, sl, :], tgt_sb[:, sl, :], tgt_sb[:, sl, :]
                )
                nc.vector.tensor_reduce(
                    ssq_t[:, sl], tsq[:, sl, :],
                    axis=mybir.AxisListType.X, op=mybir.AluOpType.add,
                )
            else:
                for j in range(j0, j0 + H):
                    nc.vector.tensor_tensor_reduce(
                        out=tsq[:, j, :],
                        in0=tgt_sb[:, j, :],
                        in1=tgt_sb[:, j, :],
                        scale=1.0,
                        scalar=0.0,
                        op0=mybir.AluOpType.mult,
                        op1=mybir.AluOpType.add,
                        accum_out=ssq_t[:, j : j + 1],
                    )
            nc.scalar.activation(tn[:, sl], ssq_t[:, sl], SQRT)
            nc.vector.reciprocal(inv_t[:, sl], tn[:, sl])
            nc.vector.tensor_scalar_mul(winv[:, sl], inv_t[:, sl], -1.0)
            for j in range(j0, j0 + H):
                nc.tensor.matmul(
                    srow_ps[:],
                    winv[:, j : j + 1],
                    tgt_sb[:, j, :],
                    start=(j == 0),
                    stop=(j == ET - 1),
                )
        nc.vector.tensor_copy(s_bf[:], srow_ps[:])
        # column-ize: s_col[a] = s[a*128:(a+1)*128]^T  via K=1 matmul
        if NO_COLMM:
            nc.vector.memset(s_colbf[:], 0.02)
        else:
            for a in range(A):
                nc.tensor.matmul(
                    scol_ps[a][:],
                    s_bf[:, a * 128 : (a + 1) * 128],
                    one_bf[:],
                    start=True,
                    stop=True,
                )
                nc.vector.tensor_copy(s_colbf[:, a : a + 1], scol_ps[a][:])
        for a in range(A):
            nc.vector.tensor_mul(
                dotw[:, a],
                eye2d,
                s_colbf[:, a : a + 1].broadcast_to([128, EYC]),
            )

        # ---- node squares (per chunk, engine per CHUNKS table)
        for name, ring, gs, sqe in CHUNKS:
            src = vt_sb[:, :, vcols(gs[0], gs[-1] + 1)]
            dst = vsq[:, :, gs[0] * G : (gs[-1] + 1) * G]
            if sqe == "S" and not SQ_DVE:
                nc.scalar.activation(dst, src, SQUARE)
            else:
                nc.vector.tensor_mul(dst, src, src)

        # ---- PE node matmuls: block-diag lhsT routes group g -> psum row
        def pair_of(g):
            return (0, g) if g < NP else (1, g - NP)

        def emit_mms(order, ps, lhs_for):
            # start/stop per psum pair follow emission order
            first_seen = {0: True, 1: True}
            remaining = {0: sum(1 for g in order if g < NP),
                         1: sum(1 for g in order if g >= NP)}
            for g in order:
                p, r = pair_of(g)
                remaining[p] -= 1
                for a in range(A):
                    nc.tensor.matmul(
                        ps[p][:],
                        lhs_for(a, r),
                        (vsq[:, a, g * G : (g + 1) * G]
                         if ps is sq_ps
                         else vt_sb[:, a, vcols(g, g + 1)]),
                        start=(first_seen[p] and a == 0),
                        stop=(remaining[p] == 0 and a == 1),
                    )
                first_seen[p] = False

        # interleave: pair-0 dots+ssq first, tail pair 0, then pair 1
        d0 = [g for g in DOT_ORDER if g < NP]
        d1 = [g for g in DOT_ORDER if g >= NP]
        q0 = [g for g in SSQ_ORDER if g < NP]
        q1 = [g for g in SSQ_ORDER if g >= NP]

        emit_mms(d0, dot_ps, lambda a, r: dotw[:, a, r * NP : (r + 1) * NP])
        emit_mms(q0, sq_ps, lambda a, r: eye2d[:, r * NP : (r + 1) * NP])
        # pair 0 tail
        nc.scalar.activation(vn[0][:], sq_ps[0][:], SQRT)
        nc.vector.reciprocal(isv[0][:], vn[0][:])
        nc.vector.tensor_mul(res[0][:], dot_ps[0][:], isv[0][:])
        nc.sync.dma_start(out_v[0:NP, :], res[0][:])

        emit_mms(d1, dot_ps, lambda a, r: dotw[:, a, r * NP : (r + 1) * NP])
        emit_mms(q1, sq_ps, lambda a, r: eye2d[:, r * NP : (r + 1) * NP])
        # pair 1 tail
        nc.scalar.activation(vn[1][:], sq_ps[1][:], SQRT)
        nc.vector.reciprocal(isv[1][:], vn[1][:])
        nc.vector.tensor_mul(res[1][:], dot_ps[1][:], isv[1][:])
        nc.sync.dma_start(out_v[NP : 2 * NP, :], res[1][:])

    nc.compile()
    return nc


def _get_nc():
    if "nc" not in _cache:
        _cache["nc"] = _build()
    return _cache["nc"]


def _host_inputs(target, node_emb):
    tgt_bf = np.ascontiguousarray(np.asarray(target, dtype=np.float32)).astype(BF)
    node_emb = np.asarray(node_emb, dtype=np.float32)

    eye = np.zeros((128, EYC), dtype=BF)
    for r in range(NP):
        eye[:, r * NP + r] = 1.0

    in_maps = []
    for c in range(N_CORES):
        shard = np.empty((NPAD, D), dtype=np.float32)
        shard[:NPC] = node_emb[c * NPC : (c + 1) * NPC]
        shard[NPC:] = node_emb[: NPAD - NPC]  # pad with real rows (no 0-norm)
        vtp = np.empty((D, NC2), dtype=BF)
        vtp[:128, 0:EYC] = eye
        vtp[128:, 0:EYC] = 0
        vtp[:, EYC:] = shard.T.astype(BF)
        in_maps.append(
            {"target": tgt_bf, "vt": np.ascontiguousarray(vtp)}
        )
    return in_maps


def run(pred, target, node_emb, trace=False, **trace_kwargs):
    """Returns (full_output [50000] f32, BassKernelResults)."""
    nc = _get_nc()
    in_maps = _host_inputs(target, node_emb)
    res = bass_utils.run_bass_kernel_spmd(
        nc, in_maps, list(range(N_CORES)), trace=trace, **trace_kwargs
    )
    parts = [res.results[c]["out"][:NPC] for c in range(N_CORES)]
    return np.concatenate(parts).astype(np.float32), res


def kernel(pred, target, node_emb):
    out, _ = run(pred, target, node_emb)
    return out


# revision 18
# speedup vs baseline: 1.0135x; 1.0135x over previous
"""ContrastiveDist kernel for TRN2 (8 NeuronCores, SPMD) -- v4.1.

out[n] = sum_e -(t_e . v_n) / (||t_e|| * ||v_n|| + eps)
       = (s . v_n) / ||v_n||      with s = -sum_e t_e / ||t_e||
(eps shifts the result by ~4e-11 relative -- dropped.)

Schedule design (from the v3/v4.0 traces):
 * ALL input DMA issues first, split across BOTH HWDGE rings (SP via
   nc.sync + ACT via nc.scalar) -- measured 300 GB/s aggregate.  eye
   rides inside the vt tensor (cols 0-48).
 * target ships entity-major [128e, 16, 256d]; per quarter (4 tiles):
   square+reduce (DVE for q0/q1, GPSIMD for q2/q3 -- gpsimd is
   otherwise idle) -> ACT Abs_reciprocal_sqrt -> DVE negate+bf16 -> PE
   matmuls with 1-column weights accumulate the s ROW [1, 256] in psum.
 * Abs_reciprocal_sqrt (|x|^-1/2, exact for our positive inputs) is in
   the abs_reciprocal_sqrt_and_small table set WITH square -> one
   ACT_TABLE_LOAD, and no bass-blocked Rsqrt and no 2.9us DVE
   RECIPROCAL in the tails.
 * s row -> per-half columns via two K=1 matmuls vs ones[1,1]; dotw =
   eye * s_col broadcast (block-diag lhsT routes group g's [1,448]
   reduction to psum row g).
 * node squares split DVE / ACT / GPSIMD per chunk; final chunks are
   1 group per ring so the post-last-byte chain is short.
 * pair tails: ACT arsqrt [7,448] then one DVE mul; out DMAs on SP.
 * fused DVE tensor_tensor_reduce is NOT used anywhere: it locks up
   the hardware (v4.0 bisect; likely why v3 shipped "batched").
"""

import os
import numpy as np
import ml_dtypes
from contextlib import ExitStack

import concourse.bacc as bacc
import concourse.bass as bass
import concourse.mybir as mybir
import concourse.tile as tile
from concourse import bass_utils

E, D = 2048, 256          # entities, embed dim
N_FULL = 50000            # total nodes
N_CORES = 8
NPC = N_FULL // N_CORES   # 6250 true nodes per core
G = 448                   # node columns per psum group (fp32 bank width)
NG = 14                   # groups per core -> NPAD = 6272
NPAD = G * NG
NP = 7                    # groups per psum pair
A = 2                     # d-halves (256 = 2*128 partitions)
ET = E // 128             # 16 entity tiles [128, 256]
EYC = NP * NP             # eye columns folded into vt (49)
NC2 = EYC + NPAD          # vt total columns
TQ = 4                    # tgt DMA chunks (quarters of 4 tiles)
H = ET // TQ
WARM_MM = 6               # PE prewarm dummy matmuls

# node chunks: (name, ring, [groups], square engine)
# ring S=SP(sync) A=ACT(scalar); square engine V=DVE S=ACT G=GPSIMD
CHUNKS = [
    ("SA", "S", [0, 1, 2, 3], "V"),
    ("XA", "A", [4, 5, 6], "S"),
    ("SB", "S", [7, 8], "G"),
    ("XB", "A", [9, 10, 11], "G"),
    ("XC", "A", [12], "S"),
    ("SC", "S", [13], "V"),
]
# PE emission order for dots / ssq (by expected data-ready time)
DOT_ORDER = [4, 5, 6, 0, 1, 2, 3, 7, 8, 9, 10, 11, 12, 13]
SSQ_ORDER = [4, 5, 6, 0, 1, 2, 3, 9, 10, 11, 7, 8, 12, 13]

F32 = mybir.dt.float32
BF16 = mybir.dt.bfloat16
BF = ml_dtypes.bfloat16
ARSQRT = mybir.ActivationFunctionType.Abs_reciprocal_sqrt
SQUARE = mybir.ActivationFunctionType.Square

_cache = {}


def _build():
    nc = bacc.Bacc(
        "TRN2",
        target_bir_lowering=False,
        debug=False,
        enable_asserts=True,
        num_devices=N_CORES,
    )
    tgt = nc.dram_tensor("target", [E, D], BF16, kind="ExternalInput").ap()
    vt = nc.dram_tensor("vt", [D, NC2], BF16, kind="ExternalInput").ap()
    out = nc.dram_tensor("out", [NG * G], F32, kind="ExternalOutput").ap()

    with tile.TileContext(nc) as tc, ExitStack() as ctx:
        tpool = ctx.enter_context(tc.tile_pool(name="tgt", bufs=1))
        vpool = ctx.enter_context(tc.tile_pool(name="v", bufs=1))
        spool = ctx.enter_context(tc.tile_pool(name="small", bufs=1))
        scr = ctx.enter_context(tc.tile_pool(name="scr", bufs=1))
        ps_w = ctx.enter_context(tc.tile_pool(name="psw", bufs=1, space="PSUM"))
        ps_sr = ctx.enter_context(tc.tile_pool(name="pssr", bufs=1, space="PSUM"))
        ps_c0 = ctx.enter_context(tc.tile_pool(name="psc0", bufs=1, space="PSUM"))
        ps_c1 = ctx.enter_context(tc.tile_pool(name="psc1", bufs=1, space="PSUM"))
        ps_da = ctx.enter_context(tc.tile_pool(name="psda", bufs=1, space="PSUM"))
        ps_db = ctx.enter_context(tc.tile_pool(name="psdb", bufs=1, space="PSUM"))
        ps_qa = ctx.enter_context(tc.tile_pool(name="psqa", bufs=1, space="PSUM"))
        ps_qb = ctx.enter_context(tc.tile_pool(name="psqb", bufs=1, space="PSUM"))

        tgt_sb = tpool.tile([128, ET, D], BF16, name="tgt_sb")
        tsq = scr.tile([128, ET, D], BF16, name="tsq")
        vt_sb = vpool.tile([128, A, NC2], BF16, name="vt_sb")
        vsq = vpool.tile([128, A, NPAD], BF16, name="vsq")

        ssq_t = spool.tile([128, ET], F32, name="ssq_t")
        inv_t = spool.tile([128, ET], F32, name="inv_t")
        winv = spool.tile([128, ET], BF16, name="winv")
        s_bf = spool.tile([1, D], BF16, name="s_bf")
        one_bf = spool.tile([1, 1], BF16, name="one_bf")
        s_colbf = spool.tile([128, A], BF16, name="s_colbf")
        dotw = spool.tile([128, A, EYC], BF16, name="dotw")
        warm_w = spool.tile([128, 1], BF16, name="warm_w")
        warm_x = spool.tile([128, G], BF16, name="warm_x")
        act_d = spool.tile([1, 1], F32, name="act_d")
        act_s = spool.tile([1, 1], F32, name="act_s")
        isv = [
            spool.tile([NP, G], F32, name="isva"),
            spool.tile([NP, G], F32, name="isvb"),
        ]
        res = [
            spool.tile([NP, G], F32, name="resa"),
            spool.tile([NP, G], F32, name="resb"),
        ]

        warm_ps = ps_w.tile([1, G], F32, name="warm_ps")
        srow_ps = ps_sr.tile([1, D], F32, name="srow_ps")
        scol_ps = [
            ps_c0.tile([128, 1], F32, name="scol0"),
            ps_c1.tile([128, 1], F32, name="scol1"),
        ]
        dot_ps = [
            ps_da.tile([NP, G], F32, name="dot_psa"),
            ps_db.tile([NP, G], F32, name="dot_psb"),
        ]
        sq_ps = [
            ps_qa.tile([NP, G], F32, name="sq_psa"),
            ps_qb.tile([NP, G], F32, name="sq_psb"),
        ]

        tgt_v = tgt.rearrange("(p j) d -> p j d", j=ET)
        vt_v = vt.rearrange("(a p) n -> p a n", p=128)
        out_v = out.rearrange("(g f) -> g f", f=G)
        eye2d = vt_sb[:, 0, 0:EYC]           # [128, 49] block-diag bf16

        def vcols(g0, g1):
            return slice(EYC + g0 * G, EYC + g1 * G)

        # ---- ALL input DMA issues first
        # SP ring: tgt quarters 0-1, eye, then SP node chunks
        for q in range(2):
            nc.sync.dma_start(
                tgt_sb[:, q * H : (q + 1) * H, :], tgt_v[:, q * H : (q + 1) * H, :]
            )
        nc.sync.dma_start(vt_sb[:, :, 0:EYC], vt_v[:, :, 0:EYC])
        # ACT ring: tgt quarters 2-3, then table preload, then node chunks
        for q in range(2, 4):
            nc.scalar.dma_start(
                tgt_sb[:, q * H : (q + 1) * H, :], tgt_v[:, q * H : (q + 1) * H, :]
            )
        nc.vector.memset(act_d[:], 1.0)
        nc.scalar.activation(act_s[:], act_d[:], ARSQRT)  # pins the table load
        for name, ring, gs, _sq in CHUNKS:
            sl = vcols(gs[0], gs[-1] + 1)
            eng = nc.sync if ring == "S" else nc.scalar
            eng.dma_start(vt_sb[:, :, sl], vt_v[:, :, sl])

        # ---- consts
        nc.vector.memset(warm_w[:], 1.0)
        nc.vector.memset(warm_x[:], 0.0)
        nc.vector.memset(one_bf[:], 1.0)

        # ---- PE prewarm (HAM clock gate wants ~3.4us of activity)
        for _ in range(WARM_MM):
            nc.tensor.matmul(warm_ps[:], warm_w[:], warm_x[:], start=True, stop=True)

        # ---- phase A: s row = -sum_e t_e/||t_e||  as [1, 256] psum
        for q in range(TQ):
            j0 = q * H
            sl = slice(j0, j0 + H)
            with tc.high_priority():
                if q < 2:
                    nc.vector.tensor_mul(
                        tsq[:, sl, :], tgt_sb[:, sl, :], tgt_sb[:, sl, :]
                    )
                    nc.vector.tensor_reduce(
                        ssq_t[:, sl], tsq[:, sl, :],
                        axis=mybir.AxisListType.X, op=mybir.AluOpType.add,
                    )
                else:
                    # ACT square + free-axis accumulate, per tile
                    for j in range(j0, j0 + H):
                        nc.scalar.activation(
                            tsq[:, j, :], tgt_sb[:, j, :], SQUARE,
                            accum_out=ssq_t[:, j : j + 1],
                        )
                nc.scalar.activation(inv_t[:, sl], ssq_t[:, sl], ARSQRT)
                nc.vector.tensor_scalar_mul(winv[:, sl], inv_t[:, sl], -1.0)
            for j in range(j0, j0 + H):
                nc.tensor.matmul(
                    srow_ps[:],
                    winv[:, j : j + 1],
                    tgt_sb[:, j, :],
                    start=(j == 0),
                    stop=(j == ET - 1),
                )
        with tc.high_priority():
            nc.vector.tensor_copy(s_bf[:], srow_ps[:])
        # column-ize: s_col[a] = s[a*128:(a+1)*128]^T  via K=1 matmul
        for a in range(A):
            nc.tensor.matmul(
                scol_ps[a][:],
                s_bf[:, a * 128 : (a + 1) * 128],
                one_bf[:],
                start=True,
                stop=True,
            )
            with tc.high_priority():
                nc.vector.tensor_copy(s_colbf[:, a : a + 1], scol_ps[a][:])
        for a in range(A):
            with tc.high_priority():
                nc.vector.tensor_mul(
                    dotw[:, a],
                    eye2d,
                    s_colbf[:, a : a + 1].broadcast_to([128, EYC]),
                )

        # ---- node squares (per chunk, engine per CHUNKS table)
        for name, ring, gs, sqe in CHUNKS:
            src = vt_sb[:, :, vcols(gs[0], gs[-1] + 1)]
            dst = vsq[:, :, gs[0] * G : (gs[-1] + 1) * G]
            if sqe == "S":
                nc.scalar.activation(dst, src, SQUARE)
            elif sqe == "G":
                nc.gpsimd.tensor_mul(dst, src, src)
            else:
                nc.vector.tensor_mul(dst, src, src)

        # ---- PE node matmuls: block-diag lhsT routes group g -> psum row
        def pair_of(g):
            return (0, g) if g < NP else (1, g - NP)

        def emit_mms(order, ps, lhs_for):
            first_seen = {0: True, 1: True}
            remaining = {0: sum(1 for g in order if g < NP),
                         1: sum(1 for g in order if g >= NP)}
            for g in order:
                p, r = pair_of(g)
                remaining[p] -= 1
                for a in range(A):
                    nc.tensor.matmul(
                        ps[p][:],
                        lhs_for(a, r),
                        (vsq[:, a, g * G : (g + 1) * G]
                         if ps is sq_ps
                         else vt_sb[:, a, vcols(g, g + 1)]),
                        start=(first_seen[p] and a == 0),
                        stop=(remaining[p] == 0 and a == 1),
                    )
                first_seen[p] = False

        d0 = [g for g in DOT_ORDER if g < NP]
        d1 = [g for g in DOT_ORDER if g >= NP]
        q0 = [g for g in SSQ_ORDER if g < NP]
        q1 = [g for g in SSQ_ORDER if g >= NP]

        emit_mms(d0, dot_ps, lambda a, r: dotw[:, a, r * NP : (r + 1) * NP])
        emit_mms(q0, sq_ps, lambda a, r: eye2d[:, r * NP : (r + 1) * NP])
        # pair 0 tail
        nc.scalar.activation(isv[0][:], sq_ps[0][:], ARSQRT)
        nc.vector.tensor_mul(res[0][:], dot_ps[0][:], isv[0][:])
        nc.sync.dma_start(out_v[0:NP, :], res[0][:])

        emit_mms(d1, dot_ps, lambda a, r: dotw[:, a, r * NP : (r + 1) * NP])
        emit_mms(q1, sq_ps, lambda a, r: eye2d[:, r * NP : (r + 1) * NP])
        # pair 1 tail
        nc.scalar.activation(isv[1][:], sq_ps[1][:], ARSQRT)
        nc.vector.tensor_mul(res[1][:], dot_ps[1][:], isv[1][:])
        nc.sync.dma_start(out_v[NP : 2 * NP, :], res[1][:])

    nc.compile()
    return nc


def _get_nc():
    if "nc" not in _cache:
        _cache["nc"] = _build()
    return _cache["nc"]


def _host_inputs(target, node_emb):
    tgt_bf = np.ascontiguousarray(np.asarray(target, dtype=np.float32)).astype(BF)
    node_emb = np.asarray(node_emb, dtype=np.float32)

    eye = np.zeros((128, EYC), dtype=BF)
    for r in range(NP):
        eye[:, r * NP + r] = 1.0

    in_maps = []
    for c in range(N_CORES):
        shard = np.empty((NPAD, D), dtype=np.float32)
        shard[:NPC] = node_emb[c * NPC : (c + 1) * NPC]
        shard[NPC:] = node_emb[: NPAD - NPC]  # pad with real rows (no 0-norm)
        vtp = np.empty((D, NC2), dtype=BF)
        vtp[:128, 0:EYC] = eye
        vtp[128:, 0:EYC] = 0
        vtp[:, EYC:] = shard.T.astype(BF)
        in_maps.append(
            {"target": tgt_bf, "vt": np.ascontiguousarray(vtp)}
        )
    return in_maps


def run(pred, target, node_emb, trace=False, **trace_kwargs):
    """Returns (full_output [50000] f32, BassKernelResults)."""
    nc = _get_nc()
    in_maps = _host_inputs(target, node_emb)
    res = bass_utils.run_bass_kernel_spmd(
        nc, in_maps, list(range(N_CORES)), trace=trace, **trace_kwargs
    )
    parts = [res.results[c]["out"][:NPC] for c in range(N_CORES)]
    return np.concatenate(parts).astype(np.float32), res


def kernel(pred, target, node_emb):
    out, _ = run(pred, target, node_emb)
    return out


# revision 19
# speedup vs baseline: 1.0583x; 1.0442x over previous
"""ContrastiveDist kernel for TRN2 (8 NeuronCores, SPMD) -- v4.2.

out[n] = sum_e -(t_e . v_n) / (||t_e|| * ||v_n|| + eps)
       = (s . v_n) / ||v_n||      with s = -sum_e t_e / ||t_e||
(eps shifts the result by ~4e-11 relative -- dropped.)

Schedule design (from the v3/v4.0/v4.1 traces):
 * THREE DMA queues: SP HWDGE (nc.sync), ACT HWDGE (nc.scalar), and the
   GPSIMD SWDGE ring (nc.gpsimd) -- SDMA engines round-robin between
   queues at packet granularity; 2 queues measured 300 GB/s aggregate.
   All input issues first; ~1.4 MB per ring.  eye rides in vt cols 0-48.
 * target entity-major [128e, 16, 256d] in 4 quarters (2 per HWDGE
   ring).  Per quarter: square (DVE for q0/q1, GPSIMD for q2/q3) ->
   DVE free-axis reduce -> ACT Abs_reciprocal_sqrt -> DVE negate+bf16
   -> PE matmuls w/ 1-column weights accumulate s ROW [1,256] in psum.
   The thin chain ops (reduce/arsqrt/negate) carry tc.high_priority;
   bulk squares do NOT (v4.1 bug: they clogged ACT ahead of DMA issues).
 * Abs_reciprocal_sqrt (|x|^-1/2, exact for positive input, measured
   +2e-5 rel err) shares its ACT table set with Square -> one
   ACT_TABLE_LOAD, no bass-blocked Rsqrt, no 2.9us DVE RECIPROCAL.
 * s row -> per-half columns via two K=1 matmuls vs ones[1,1]; dotw =
   eye * s_col broadcast (block-diag lhsT routes group g's [1,448]
   reduction to psum row g).
 * node squares: DVE 0.47us/group, ACT 0.75, GPSIMD 1.9 -- assigned so
   every group's square lands before its ssq matmul slot; the three
   final chunks are 1 group each on different rings.
 * pair tails: ACT arsqrt [7,448] then one DVE mul; out DMAs on SP.
 * fused DVE tensor_tensor_reduce is NOT used anywhere: it locks up
   the hardware (v4.0 bisect).  ACT Square+accum_out works but costs
   769ns/tile (ACTIVATE + ACTIVATION_READ_ACCUMULATOR) -- not used.
"""

import numpy as np
import ml_dtypes
from contextlib import ExitStack

import concourse.bacc as bacc
import concourse.bass as bass
import concourse.mybir as mybir
import concourse.tile as tile
from concourse import bass_utils

E, D = 2048, 256          # entities, embed dim
N_FULL = 50000            # total nodes
N_CORES = 8
NPC = N_FULL // N_CORES   # 6250 true nodes per core
G = 448                   # node columns per psum group (fp32 bank width)
NG = 14                   # groups per core -> NPAD = 6272
NPAD = G * NG
NP = 7                    # groups per psum pair
A = 2                     # d-halves (256 = 2*128 partitions)
ET = E // 128             # 16 entity tiles [128, 256]
EYC = NP * NP             # eye columns folded into vt (49)
NC2 = EYC + NPAD          # vt total columns
TQ = 4                    # tgt DMA chunks (quarters of 4 tiles)
H = ET // TQ
WARM_MM = 6               # PE prewarm dummy matmuls

# node chunks: (name, ring, [groups], square engine)
# ring S=SP(sync) A=ACT(scalar) G=GPSIMD(swdge)
# square engine V=DVE S=ACT G=GPSIMD
CHUNKS = [
    ("SA", "S", [0, 1, 2], "S"),
    ("XA", "A", [3, 4, 5], "S"),
    ("GA", "G", [6, 7, 8], None),   # per-group squares, see SQ_ENG
    ("GB", "G", [9, 10], "V"),
    ("GC", "G", [11], "V"),
    ("SD", "S", [12], "S"),
    ("XD", "A", [13], "V"),
]
SQ_ENG = {6: "G", 7: "V", 8: "V"}   # GA per-group square engines
# PE emission order for dots / ssq (by expected data/square readiness)
DOT_ORDER = [6, 7, 8, 9, 10, 0, 1, 2, 3, 4, 5, 11, 12, 13]
SSQ_ORDER = [7, 8, 6, 9, 10, 0, 1, 2, 3, 4, 5, 11, 13, 12]

F32 = mybir.dt.float32
BF16 = mybir.dt.bfloat16
BF = ml_dtypes.bfloat16
ARSQRT = mybir.ActivationFunctionType.Abs_reciprocal_sqrt
SQUARE = mybir.ActivationFunctionType.Square

_cache = {}


def _build():
    nc = bacc.Bacc(
        "TRN2",
        target_bir_lowering=False,
        debug=False,
        enable_asserts=True,
        num_devices=N_CORES,
    )
    tgt = nc.dram_tensor("target", [E, D], BF16, kind="ExternalInput").ap()
    vt = nc.dram_tensor("vt", [D, NC2], BF16, kind="ExternalInput").ap()
    out = nc.dram_tensor("out", [NG * G], F32, kind="ExternalOutput").ap()

    with tile.TileContext(nc) as tc, ExitStack() as ctx:
        tpool = ctx.enter_context(tc.tile_pool(name="tgt", bufs=1))
        vpool = ctx.enter_context(tc.tile_pool(name="v", bufs=1))
        spool = ctx.enter_context(tc.tile_pool(name="small", bufs=1))
        scr = ctx.enter_context(tc.tile_pool(name="scr", bufs=1))
        ps_w = ctx.enter_context(tc.tile_pool(name="psw", bufs=1, space="PSUM"))
        ps_sr = ctx.enter_context(tc.tile_pool(name="pssr", bufs=1, space="PSUM"))
        ps_c0 = ctx.enter_context(tc.tile_pool(name="psc0", bufs=1, space="PSUM"))
        ps_c1 = ctx.enter_context(tc.tile_pool(name="psc1", bufs=1, space="PSUM"))
        ps_da = ctx.enter_context(tc.tile_pool(name="psda", bufs=1, space="PSUM"))
        ps_db = ctx.enter_context(tc.tile_pool(name="psdb", bufs=1, space="PSUM"))
        ps_qa = ctx.enter_context(tc.tile_pool(name="psqa", bufs=1, space="PSUM"))
        ps_qb = ctx.enter_context(tc.tile_pool(name="psqb", bufs=1, space="PSUM"))

        tgt_sb = tpool.tile([128, ET, D], BF16, name="tgt_sb")
        tsq = scr.tile([128, ET, D], BF16, name="tsq")
        vt_sb = vpool.tile([128, A, NC2], BF16, name="vt_sb")
        vsq = vpool.tile([128, A, NPAD], BF16, name="vsq")

        ssq_t = spool.tile([128, ET], F32, name="ssq_t")
        inv_t = spool.tile([128, ET], F32, name="inv_t")
        winv = spool.tile([128, ET], BF16, name="winv")
        s_bf = spool.tile([1, D], BF16, name="s_bf")
        one_bf = spool.tile([1, 1], BF16, name="one_bf")
        s_colbf = spool.tile([128, A], BF16, name="s_colbf")
        dotw = spool.tile([128, A, EYC], BF16, name="dotw")
        warm_w = spool.tile([128, 1], BF16, name="warm_w")
        warm_x = spool.tile([128, G], BF16, name="warm_x")
        act_d = spool.tile([1, 1], F32, name="act_d")
        act_s = spool.tile([1, 1], F32, name="act_s")
        isv = [
            spool.tile([NP, G], F32, name="isva"),
            spool.tile([NP, G], F32, name="isvb"),
        ]
        res = [
            spool.tile([NP, G], F32, name="resa"),
            spool.tile([NP, G], F32, name="resb"),
        ]

        warm_ps = ps_w.tile([1, G], F32, name="warm_ps")
        srow_ps = ps_sr.tile([1, D], F32, name="srow_ps")
        scol_ps = [
            ps_c0.tile([128, 1], F32, name="scol0"),
            ps_c1.tile([128, 1], F32, name="scol1"),
        ]
        dot_ps = [
            ps_da.tile([NP, G], F32, name="dot_psa"),
            ps_db.tile([NP, G], F32, name="dot_psb"),
        ]
        sq_ps = [
            ps_qa.tile([NP, G], F32, name="sq_psa"),
            ps_qb.tile([NP, G], F32, name="sq_psb"),
        ]

        tgt_v = tgt.rearrange("(p j) d -> p j d", j=ET)
        vt_v = vt.rearrange("(a p) n -> p a n", p=128)
        out_v = out.rearrange("(g f) -> g f", f=G)
        eye2d = vt_sb[:, 0, 0:EYC]           # [128, 49] block-diag bf16

        def vcols(g0, g1):
            return slice(EYC + g0 * G, EYC + g1 * G)

        ring_eng = {"S": nc.sync, "A": nc.scalar, "G": nc.gpsimd}

        # ---- ALL input DMA issues first
        for q in range(2):
            nc.sync.dma_start(
                tgt_sb[:, q * H : (q + 1) * H, :], tgt_v[:, q * H : (q + 1) * H, :]
            )
        nc.sync.dma_start(vt_sb[:, :, 0:EYC], vt_v[:, :, 0:EYC])
        for q in range(2, 4):
            nc.scalar.dma_start(
                tgt_sb[:, q * H : (q + 1) * H, :], tgt_v[:, q * H : (q + 1) * H, :]
            )
        nc.vector.memset(act_d[:], 1.0)
        nc.scalar.activation(act_s[:], act_d[:], ARSQRT)  # pins the table load
        for name, ring, gs, _sq in CHUNKS:
            sl = vcols(gs[0], gs[-1] + 1)
            ring_eng[ring].dma_start(vt_sb[:, :, sl], vt_v[:, :, sl])

        # ---- consts
        nc.vector.memset(warm_w[:], 1.0)
        nc.vector.memset(warm_x[:], 0.0)
        nc.vector.memset(one_bf[:], 1.0)

        # ---- PE prewarm (HAM clock gate wants ~3.4us of activity)
        for _ in range(WARM_MM):
            nc.tensor.matmul(warm_ps[:], warm_w[:], warm_x[:], start=True, stop=True)

        # ---- phase A: s row = -sum_e t_e/||t_e||  as [1, 256] psum
        for q in range(TQ):
            j0 = q * H
            sl = slice(j0, j0 + H)
            sq_eng = nc.vector if q < 2 else nc.gpsimd
            sq_eng.tensor_mul(tsq[:, sl, :], tgt_sb[:, sl, :], tgt_sb[:, sl, :])
            with tc.high_priority():
                nc.vector.tensor_reduce(
                    ssq_t[:, sl], tsq[:, sl, :],
                    axis=mybir.AxisListType.X, op=mybir.AluOpType.add,
                )
                nc.scalar.activation(inv_t[:, sl], ssq_t[:, sl], ARSQRT)
                nc.vector.tensor_scalar_mul(winv[:, sl], inv_t[:, sl], -1.0)
            for j in range(j0, j0 + H):
                nc.tensor.matmul(
                    srow_ps[:],
                    winv[:, j : j + 1],
                    tgt_sb[:, j, :],
                    start=(j == 0),
                    stop=(j == ET - 1),
                )
        with tc.high_priority():
            nc.vector.tensor_copy(s_bf[:], srow_ps[:])
        for a in range(A):
            nc.tensor.matmul(
                scol_ps[a][:],
                s_bf[:, a * 128 : (a + 1) * 128],
                one_bf[:],
                start=True,
                stop=True,
            )
            with tc.high_priority():
                nc.vector.tensor_copy(s_colbf[:, a : a + 1], scol_ps[a][:])
        for a in range(A):
            with tc.high_priority():
                nc.vector.tensor_mul(
                    dotw[:, a],
                    eye2d,
                    s_colbf[:, a : a + 1].broadcast_to([128, EYC]),
                )

        # ---- node squares (engine per CHUNKS table / SQ_ENG for GA)
        sq_eng_map = {"V": nc.vector, "S": nc.scalar, "G": nc.gpsimd}

        def emit_square(g0, g1, eng):
            src = vt_sb[:, :, vcols(g0, g1)]
            dst = vsq[:, :, g0 * G : g1 * G]
            if eng == "S":
                nc.scalar.activation(dst, src, SQUARE)
            else:
                sq_eng_map[eng].tensor_mul(dst, src, src)

        for name, ring, gs, sqe in CHUNKS:
            if sqe is None:
                for g in gs:
                    emit_square(g, g + 1, SQ_ENG[g])
            else:
                emit_square(gs[0], gs[-1] + 1, sqe)

        # ---- PE node matmuls: block-diag lhsT routes group g -> psum row
        def pair_of(g):
            return (0, g) if g < NP else (1, g - NP)

        def emit_mms(order, ps, lhs_for):
            first_seen = {0: True, 1: True}
            remaining = {0: sum(1 for g in order if g < NP),
                         1: sum(1 for g in order if g >= NP)}
            for g in order:
                p, r = pair_of(g)
                remaining[p] -= 1
                for a in range(A):
                    nc.tensor.matmul(
                        ps[p][:],
                        lhs_for(a, r),
                        (vsq[:, a, g * G : (g + 1) * G]
                         if ps is sq_ps
                         else vt_sb[:, a, vcols(g, g + 1)]),
                        start=(first_seen[p] and a == 0),
                        stop=(remaining[p] == 0 and a == 1),
                    )
                first_seen[p] = False

        emit_mms(DOT_ORDER, dot_ps, lambda a, r: dotw[:, a, r * NP : (r + 1) * NP])
        emit_mms(SSQ_ORDER, sq_ps, lambda a, r: eye2d[:, r * NP : (r + 1) * NP])
        # tails (pair 0 then pair 1; scheduler runs each when its pair closes)
        for p in range(2):
            nc.scalar.activation(isv[p][:], sq_ps[p][:], ARSQRT)
            nc.vector.tensor_mul(res[p][:], dot_ps[p][:], isv[p][:])
            nc.sync.dma_start(out_v[p * NP : (p + 1) * NP, :], res[p][:])

    nc.compile()
    return nc


def _get_nc():
    if "nc" not in _cache:
        _cache["nc"] = _build()
    return _cache["nc"]


def _host_inputs(target, node_emb):
    tgt_bf = np.ascontiguousarray(np.asarray(target, dtype=np.float32)).astype(BF)
    node_emb = np.asarray(node_emb, dtype=np.float32)

    eye = np.zeros((128, EYC), dtype=BF)
    for r in range(NP):
        eye[:, r * NP + r] = 1.0

    in_maps = []
    for c in range(N_CORES):
        shard = np.empty((NPAD, D), dtype=np.float32)
        shard[:NPC] = node_emb[c * NPC : (c + 1) * NPC]
        shard[NPC:] = node_emb[: NPAD - NPC]  # pad with real rows (no 0-norm)
        vtp = np.empty((D, NC2), dtype=BF)
        vtp[:128, 0:EYC] = eye
        vtp[128:, 0:EYC] = 0
        vtp[:, EYC:] = shard.T.astype(BF)
        in_maps.append(
            {"target": tgt_bf, "vt": np.ascontiguousarray(vtp)}
        )
    return in_maps


def run(pred, target, node_emb, trace=False, **trace_kwargs):
    """Returns (full_output [50000] f32, BassKernelResults)."""
    nc = _get_nc()
    in_maps = _host_inputs(target, node_emb)
    res = bass_utils.run_bass_kernel_spmd(
        nc, in_maps, list(range(N_CORES)), trace=trace, **trace_kwargs
    )
    parts = [res.results[c]["out"][:NPC] for c in range(N_CORES)]
    return np.concatenate(parts).astype(np.float32), res


def kernel(pred, target, node_emb):
    out, _ = run(pred, target, node_emb)
    return out


# revision 20
# speedup vs baseline: 1.2262x; 1.1587x over previous
"""ContrastiveDist kernel for TRN2 (8 NeuronCores, SPMD) -- v4.3.

out[n] = sum_e -(t_e . v_n) / (||t_e|| * ||v_n|| + eps)
       = (s . v_n) / ||v_n||      with s = -sum_e t_e / ||t_e||
(eps shifts the result by ~4e-11 relative -- dropped.)

Schedule design (from the v3/v4.0/v4.1/v4.2 traces):
 * THREE DMA queues: SP HWDGE (nc.sync), ACT HWDGE (nc.scalar), GPSIMD
   SWDGE (nc.gpsimd); ~285-300 GB/s aggregate.  All input issues first.
   tgt quarters head the two HWDGE rings; the eye block rides with node
   block 0 as the GP ring's first chunk (v4.2's standalone 98B-packet
   eye DMA stalled the SP ring).  Node blocks are laid out so every DMA
   chunk is a contiguous column range.
 * target entity-major [128e, 16, 256d] in 4 quarters.  Per quarter:
   square (DVE q0/q1, GPSIMD q2/q3) -> DVE free-axis reduce -> ACT
   Abs_reciprocal_sqrt with BF16 output (winv = +1/||t||, no DVE negate
   -- the sign folds into the ACT Copy(scale=-1) psum->sbuf column
   copies) -> PE matmuls w/ 1-col weights accumulate s ROW [1,256].
 * Abs_reciprocal_sqrt (|x|^-1/2, exact for positive input, measured
   +2e-5 rel err vs exact rsqrt) shares its ACT table with Square and
   Copy -> one ACT_TABLE_LOAD, no bass-blocked Rsqrt, no 2.9us DVE
   RECIPROCAL tails.
 * tile_wait_until logical timestamps pin each engine's static stream
   order to the EXPECTED data-arrival timeline (the Tile scheduler's
   own cost model mispredicts DMA arrivals and had reordered bulk
   squares/reduces ahead of latency-critical chain ops in v4.1/v4.2).
 * pair tails: ACT arsqrt [7,448] then one DVE mul; out DMAs on SP.
 * fused DVE tensor_tensor_reduce is NOT used anywhere: it locks up
   the hardware (v4.0 bisect).
"""

import numpy as np
import ml_dtypes
from contextlib import ExitStack

import concourse.bacc as bacc
import concourse.bass as bass
import concourse.mybir as mybir
import concourse.tile as tile
from concourse import bass_utils

E, D = 2048, 256          # entities, embed dim
N_FULL = 50000            # total nodes
N_CORES = 8
NPC = N_FULL // N_CORES   # 6250 true nodes per core
G = 448                   # node columns per psum group (fp32 bank width)
NG = 14                   # node column blocks -> NPAD = 6272
NPAD = G * NG
NP = 7                    # blocks per psum pair
A = 2                     # d-halves (256 = 2*128 partitions)
ET = E // 128             # 16 entity tiles [128, 256]
EYC = NP * NP             # eye columns at the head of vt (49)
NC2 = EYC + NPAD          # vt total columns
TQ = 4                    # tgt DMA chunks (quarters of 4 tiles)
H = ET // TQ
WARM_MM = 6               # PE prewarm dummy matmuls

# node chunks: (ring, [blocks], square engine, est data-arrival us)
# ring S=SP(sync) A=ACT(scalar) G=GPSIMD(swdge); sq V=DVE S=ACT G=GP
CHUNKS = [
    ("G", [0], "G", 4.5),          # GE: eye + block 0
    ("G", [1, 2], "V", 7.5),       # GA
    ("G", [3, 4], "V", 10.5),      # GB
    ("G", [5], "V", 12.0),         # GC
    ("S", [6, 7, 8], "S", 14.0),   # SA (per-block ACT squares)
    ("A", [9, 10, 11], "V", 14.0), # XA
    ("S", [12], "V", 15.5),        # SD
    ("A", [13], "V", 16.0),        # XD
]
# PE emission order for dots / ssq mms (by expected readiness)
DOT_ORDER = [0, 1, 2, 3, 4, 5, 6, 7, 8, 9, 10, 11, 12, 13]
SSQ_ORDER = [0, 1, 2, 3, 4, 5, 6, 9, 10, 11, 7, 8, 12, 13]
ARRIVE = {}
for _ring, _bs, _sq, _t in CHUNKS:
    for _b in _bs:
        ARRIVE[_b] = _t

F32 = mybir.dt.float32
BF16 = mybir.dt.bfloat16
BF = ml_dtypes.bfloat16
ARSQRT = mybir.ActivationFunctionType.Abs_reciprocal_sqrt
SQUARE = mybir.ActivationFunctionType.Square
COPY = mybir.ActivationFunctionType.Copy

_cache = {}


def _build():
    nc = bacc.Bacc(
        "TRN2",
        target_bir_lowering=False,
        debug=False,
        enable_asserts=True,
        num_devices=N_CORES,
    )
    tgt = nc.dram_tensor("target", [E, D], BF16, kind="ExternalInput").ap()
    vt = nc.dram_tensor("vt", [D, NC2], BF16, kind="ExternalInput").ap()
    out = nc.dram_tensor("out", [NG * G], F32, kind="ExternalOutput").ap()

    with tile.TileContext(nc) as tc, ExitStack() as ctx:
        tpool = ctx.enter_context(tc.tile_pool(name="tgt", bufs=1))
        vpool = ctx.enter_context(tc.tile_pool(name="v", bufs=1))
        spool = ctx.enter_context(tc.tile_pool(name="small", bufs=1))
        scr = ctx.enter_context(tc.tile_pool(name="scr", bufs=1))
        ps_w = ctx.enter_context(tc.tile_pool(name="psw", bufs=1, space="PSUM"))
        ps_sr = ctx.enter_context(tc.tile_pool(name="pssr", bufs=1, space="PSUM"))
        ps_c0 = ctx.enter_context(tc.tile_pool(name="psc0", bufs=1, space="PSUM"))
        ps_c1 = ctx.enter_context(tc.tile_pool(name="psc1", bufs=1, space="PSUM"))
        ps_da = ctx.enter_context(tc.tile_pool(name="psda", bufs=1, space="PSUM"))
        ps_db = ctx.enter_context(tc.tile_pool(name="psdb", bufs=1, space="PSUM"))
        ps_qa = ctx.enter_context(tc.tile_pool(name="psqa", bufs=1, space="PSUM"))
        ps_qb = ctx.enter_context(tc.tile_pool(name="psqb", bufs=1, space="PSUM"))

        tgt_sb = tpool.tile([128, ET, D], BF16, name="tgt_sb")
        tsq = scr.tile([128, ET, D], BF16, name="tsq")
        vt_sb = vpool.tile([128, A, NC2], BF16, name="vt_sb")
        vsq = vpool.tile([128, A, NPAD], BF16, name="vsq")

        ssq_t = spool.tile([128, ET], F32, name="ssq_t")
        winv = spool.tile([128, ET], BF16, name="winv")
        s_bf = spool.tile([1, D], BF16, name="s_bf")
        one_bf = spool.tile([1, 1], BF16, name="one_bf")
        s_colbf = spool.tile([128, A], BF16, name="s_colbf")
        dotw = spool.tile([128, A, EYC], BF16, name="dotw")
        warm_w = spool.tile([128, 1], BF16, name="warm_w")
        warm_x = spool.tile([128, G], BF16, name="warm_x")
        act_d = spool.tile([1, 1], F32, name="act_d")
        act_s = spool.tile([1, 1], F32, name="act_s")
        isv = [
            spool.tile([NP, G], F32, name="isva"),
            spool.tile([NP, G], F32, name="isvb"),
        ]
        res = [
            spool.tile([NP, G], F32, name="resa"),
            spool.tile([NP, G], F32, name="resb"),
        ]

        warm_ps = ps_w.tile([1, G], F32, name="warm_ps")
        srow_ps = ps_sr.tile([1, D], F32, name="srow_ps")
        scol_ps = [
            ps_c0.tile([128, 1], F32, name="scol0"),
            ps_c1.tile([128, 1], F32, name="scol1"),
        ]
        dot_ps = [
            ps_da.tile([NP, G], F32, name="dot_psa"),
            ps_db.tile([NP, G], F32, name="dot_psb"),
        ]
        sq_ps = [
            ps_qa.tile([NP, G], F32, name="sq_psa"),
            ps_qb.tile([NP, G], F32, name="sq_psb"),
        ]

        tgt_v = tgt.rearrange("(p j) d -> p j d", j=ET)
        vt_v = vt.rearrange("(a p) n -> p a n", p=128)
        out_v = out.rearrange("(g f) -> g f", f=G)
        eye2d = vt_sb[:, 0, 0:EYC]           # [128, 49] block-diag bf16

        def W(us):
            return tc.tile_wait_until(us / 1000.0)

        def vcols(b0, b1):
            return slice(EYC + b0 * G, EYC + b1 * G)

        ring_eng = {"S": nc.sync, "A": nc.scalar, "G": nc.gpsimd}
        sq_eng = {"V": nc.vector, "G": nc.gpsimd}

        # ---- ALL input DMA issues first
        for q in range(2):
            nc.sync.dma_start(
                tgt_sb[:, q * H : (q + 1) * H, :], tgt_v[:, q * H : (q + 1) * H, :]
            )
        for q in range(2, 4):
            nc.scalar.dma_start(
                tgt_sb[:, q * H : (q + 1) * H, :], tgt_v[:, q * H : (q + 1) * H, :]
            )
        nc.vector.memset(act_d[:], 1.0)
        nc.scalar.activation(act_s[:], act_d[:], ARSQRT)  # pins the table load
        for i, (ring, bs, _sq, _t) in enumerate(CHUNKS):
            lo = 0 if i == 0 else EYC + bs[0] * G      # chunk 0 includes eye
            hi = EYC + (bs[-1] + 1) * G
            sl = slice(lo, hi)
            ring_eng[ring].dma_start(vt_sb[:, :, sl], vt_v[:, :, sl])

        # ---- consts
        nc.vector.memset(warm_w[:], 1.0)
        nc.vector.memset(warm_x[:], 0.0)
        nc.vector.memset(one_bf[:], 1.0)

        # ---- PE prewarm (HAM clock gate wants ~3.4us of activity)
        for _ in range(WARM_MM):
            nc.tensor.matmul(warm_ps[:], warm_w[:], warm_x[:], start=True, stop=True)

        # ---- phase A: s row = +sum_e t_e/||t_e||  as [1, 256] psum
        # (sign flips in the scol copies)
        for q in range(TQ):
            j0 = q * H
            sl = slice(j0, j0 + H)
            eng = nc.vector if q < 2 else nc.gpsimd
            eng.tensor_mul(tsq[:, sl, :], tgt_sb[:, sl, :], tgt_sb[:, sl, :])
            with tc.high_priority():
                nc.vector.tensor_reduce(
                    ssq_t[:, sl], tsq[:, sl, :],
                    axis=mybir.AxisListType.X, op=mybir.AluOpType.add,
                )
                nc.scalar.activation(winv[:, sl], ssq_t[:, sl], ARSQRT)
            for j in range(j0, j0 + H):
                nc.tensor.matmul(
                    srow_ps[:],
                    winv[:, j : j + 1],
                    tgt_sb[:, j, :],
                    start=(j == 0),
                    stop=(j == ET - 1),
                )
        with tc.high_priority():
            nc.scalar.activation(s_bf[:], srow_ps[:], COPY)
        for a in range(A):
            nc.tensor.matmul(
                scol_ps[a][:],
                s_bf[:, a * 128 : (a + 1) * 128],
                one_bf[:],
                start=True,
                stop=True,
            )
            with tc.high_priority():
                nc.scalar.activation(
                    s_colbf[:, a : a + 1], scol_ps[a][:], COPY, scale=-1.0
                )
        for a in range(A):
            with tc.high_priority():
                nc.vector.tensor_mul(
                    dotw[:, a],
                    eye2d,
                    s_colbf[:, a : a + 1].broadcast_to([128, EYC]),
                )

        # ---- node squares (engine + logical timestamp per chunk)
        for ring, bs, sqe, t_arr in CHUNKS:
            if sqe == "S":
                for b in bs:   # per-block ACT squares (pair0 closes on b6)
                    with W(t_arr + 0.2):
                        nc.scalar.activation(
                            vsq[:, :, b * G : (b + 1) * G],
                            vt_sb[:, :, vcols(b, b + 1)],
                            SQUARE,
                        )
            else:
                with W(t_arr + 0.2):
                    sq_eng[sqe].tensor_mul(
                        vsq[:, :, bs[0] * G : (bs[-1] + 1) * G],
                        vt_sb[:, :, vcols(bs[0], bs[-1] + 1)],
                        vt_sb[:, :, vcols(bs[0], bs[-1] + 1)],
                    )

        # ---- PE node matmuls: block-diag lhsT routes block b -> psum row
        def pair_of(b):
            return (0, b) if b < NP else (1, b - NP)

        def emit_mms(order, ps, lhs_for, t_of):
            first_seen = {0: True, 1: True}
            remaining = {0: sum(1 for b in order if b < NP),
                         1: sum(1 for b in order if b >= NP)}
            for b in order:
                p, r = pair_of(b)
                remaining[p] -= 1
                with W(t_of(b)):
                    for a in range(A):
                        nc.tensor.matmul(
                            ps[p][:],
                            lhs_for(a, r),
                            (vsq[:, a, b * G : (b + 1) * G]
                             if ps is sq_ps
                             else vt_sb[:, a, vcols(b, b + 1)]),
                            start=(first_seen[p] and a == 0),
                            stop=(remaining[p] == 0 and a == 1),
                        )
                first_seen[p] = False

        S_READY = 12.6
        emit_mms(
            DOT_ORDER, dot_ps,
            lambda a, r: dotw[:, a, r * NP : (r + 1) * NP],
            lambda b: max(S_READY, ARRIVE[b] + 0.2),
        )
        emit_mms(
            SSQ_ORDER, sq_ps,
            lambda a, r: eye2d[:, r * NP : (r + 1) * NP],
            lambda b: ARRIVE[b] + 0.2 + (0.9 if b in (6, 7, 8) else 0.6),
        )
        # tails (pair 0 then pair 1)
        for p, t_tail in ((0, 15.2), (1, 17.0)):
            with W(t_tail):
                nc.scalar.activation(isv[p][:], sq_ps[p][:], ARSQRT)
                nc.vector.tensor_mul(res[p][:], dot_ps[p][:], isv[p][:])
                nc.sync.dma_start(out_v[p * NP : (p + 1) * NP, :], res[p][:])

    nc.compile()
    return nc


def _get_nc():
    if "nc" not in _cache:
        _cache["nc"] = _build()
    return _cache["nc"]


def _host_inputs(target, node_emb):
    tgt_bf = np.ascontiguousarray(np.asarray(target, dtype=np.float32)).astype(BF)
    node_emb = np.asarray(node_emb, dtype=np.float32)

    eye = np.zeros((128, EYC), dtype=BF)
    for r in range(NP):
        eye[:, r * NP + r] = 1.0

    in_maps = []
    for c in range(N_CORES):
        shard = np.empty((NPAD, D), dtype=np.float32)
        shard[:NPC] = node_emb[c * NPC : (c + 1) * NPC]
        shard[NPC:] = node_emb[: NPAD - NPC]  # pad with real rows (no 0-norm)
        vtp = np.empty((D, NC2), dtype=BF)
        vtp[:128, 0:EYC] = eye
        vtp[128:, 0:EYC] = 0
        vtp[:, EYC:] = shard.T.astype(BF)
        in_maps.append(
            {"target": tgt_bf, "vt": np.ascontiguousarray(vtp)}
        )
    return in_maps


def run(pred, target, node_emb, trace=False, **trace_kwargs):
    """Returns (full_output [50000] f32, BassKernelResults)."""
    nc = _get_nc()
    in_maps = _host_inputs(target, node_emb)
    res = bass_utils.run_bass_kernel_spmd(
        nc, in_maps, list(range(N_CORES)), trace=trace, **trace_kwargs
    )
    parts = [res.results[c]["out"][:NPC] for c in range(N_CORES)]
    return np.concatenate(parts).astype(np.float32), res


def kernel(pred, target, node_emb):
    out, _ = run(pred, target, node_emb)
    return out
